# revision 30
# baseline (speedup 1.0000x reference)
"""Multi-head causal self-attention on 8 Trainium2 NeuronCores.

Problem: X[4,2048,1024], per-head Wq/Wk/Wv[16,1024,64], Wo[1024,1024], bo[1024].
    out = OutProj(concat_heads(softmax_causal(Q K^T / 8) V))

Sharding: 8 cores = 4 batches x 2 head-groups (8 heads each). Each core
computes its batch's attention for its 8 heads plus the partial output
projection over its 512 concat features; host sums the two partials per
batch and adds the bias.

Per-core kernel (matmul operands in fp16 — 1 col/cycle on TensorE with
fp32 PSUM accumulation; softmax runs in the transposed
"feature-on-partition" space so its reduction lands on the free dim):
  qT/kT per head-pair  [128, T]  = Wpair^T  x  X^T
  v    per s-tile      [128, 8*65] = X^T^T  x  Wv_all (65th col set to 1)
  ST block [s=128, t=512] = kT_slice^T @ qT_slice   (row-packed head pairs:
     the two 64-row tiles share one fused LDWEIGHTS and stream their
     moving operands CONCURRENTLY — disjoint SBUF partitions + disjoint
     PSUM banks — so a score pair costs ~nv cycles, not 2*nv)
  expST = exp(ST/8) (ScalarE), causal-masked via tri multiply
  avT [65, 512] += [V|1]^T @ expST   -> rows 0:64 = (A@V)^T, row 64 = sums
  normalize via 1/sums broadcast and write concatT
  partial = concatT^T @ WoST  (accumulated over 4 feature chunks)

Schedule (v2): tt-MAJOR — for each 512-wide query tile tt, all four head
pairs run their attention segment back-to-back.  All pairs' Q/K live in
SBUF simultaneously, so the output projection for query block tt unlocks
as soon as phase tt completes (25/50/75/100% marks) instead of piling
into the last quarter.  Fill work (later-phase projections, V tail,
out-proj groups) is interleaved into the attention stream under a
simple clock model of PE vs ScalarE so the in-order PE queue never
head-of-line blocks on an exp that hasn't fired: per si step the AV
matmuls trail the score pair by one step, and filler is popped until
the PE clock catches the predicted exp completion.
"""

import os
import sys

for _p in ("/opt/trn_rl_repo", "/root/.axon_site/_ro/trn_rl_repo"):
    if os.path.isdir(_p) and _p not in sys.path:
        sys.path.append(_p)

import numpy as np

import concourse.mybir as mybir
import concourse.tile as tile
from concourse import bacc

B, T, D, H, K = 4, 2048, 1024, 16, 64
HG = 8          # heads per core
NPAIR = 4       # head pairs per core
P = 128
DC = D // P     # 8 contraction chunks for the projections
NS = T // P     # 16 key tiles
NT = T // 512   # 4 query tiles of 512
F32 = mybir.dt.float32
F16 = mybir.dt.float16

# clock-model constants (ns), calibrated from the v1 trace
MM_NS = 216.0 / 512.0      # per streamed column, 512-col mm ~216ns cadence
PAIR_FIX = 100.0           # extra fixed cost of a score-pair issue
EXP_COL = 0.87             # ScalarE ns per column
EXP_FIX = 260.0            # ScalarE per-activation overhead
EXP_LAG = 220.0            # sem propagation mm-done -> exp start


def build_module():
    nc = bacc.Bacc("TRN2")
    XT = nc.dram_tensor("xt", [D, T], F16, kind="ExternalInput").ap()
    WQ = nc.dram_tensor("wq", [NPAIR, D, P], F16, kind="ExternalInput").ap()
    WK = nc.dram_tensor("wk", [NPAIR, D, P], F16, kind="ExternalInput").ap()
    WV = nc.dram_tensor("wv", [D, HG * K], F16, kind="ExternalInput").ap()
    WO = nc.dram_tensor("wo", [HG * K, D], F16, kind="ExternalInput").ap()
    OUT = nc.dram_tensor("out", [T, D], F16, kind="ExternalOutput").ap()

    with tile.TileContext(nc) as tc:
        with tc.tile_pool(name="persist", bufs=1) as pp:
            xt_sb = pp.tile([P, DC, T], F16)            # X^T, 32 KB/partition
            v_sb = pp.tile([P, NS, HG * (K + 1)], F16)  # V + ones col per head
            concat_sb = pp.tile([P, NPAIR, T], F16)     # concat(heads)^T
            tri_sb = pp.tile([P, P], F16)   # causal triangle: 1 where x >= p
            warm_sb = pp.tile([P, 512], F16)
            wo_sb = pp.tile([P, NPAIR, D], F16)
            wv_sb = pp.tile([P, DC, HG * K], F16)
            wq_sb = [pp.tile([P, DC, P], F16, name=f"wq{p}") for p in range(NPAIR)]
            wk_sb = [pp.tile([P, DC, P], F16, name=f"wk{p}") for p in range(NPAIR)]
            q_sb = [pp.tile([P, T], F16, name=f"q{p}") for p in range(NPAIR)]
            k_sb = [pp.tile([P, T], F16, name=f"k{p}") for p in range(NPAIR)]

            xt_r = XT.rearrange("(c p) t -> c p t", p=P)
            xt_p = XT.rearrange("(c p) t -> p c t", p=P)
            wv_p = WV.rearrange("(c p) n -> p c n", p=P)
            wo_p = WO.rearrange("(s p) o -> p s o", p=P)

            # ---- DMA priority emission -------------------------------
            # Uniform per-chunk 2D transfers (contiguous per-partition
            # source rows) in strict need-order per queue — mixed sizes
            # and 3D patterns make transfer completion and the trigger
            # instructions (semaphore-pool reuse) unpredictable.
            # sync/gpsimd: X b0 split even/odd, Wq0/Wk0, Wq1/Wk1, Wv
            # split, Wq2/Wk2, Wq3/Wk3, X b1/b2 halves.  scalar: only
            # late-need bytes (Wo, X b3) so it stays exp-pure after ~15us.
            # Critical set first — X block 0 + Wq0/Wk0 (1.5 MB).  The DMA
            # engines are shared across queues and split bandwidth over
            # everything in flight, so bulk transfers are gated behind
            # the critical set with artificial WAR deps (a dummy Vector
            # read of each bulk destination that itself depends on the
            # critical transfers landing).
            # tier 1 (in flight alone): X b0 on sync/gpsimd, Wq0/Wk0 on
            # scalar (scalar then goes exp-pure; its later transfers are
            # emitted in-stream at segment boundaries).
            for c in range(0, DC, 2):
                nc.sync.dma_start(out=xt_sb[:, c, 0:512], in_=xt_r[c][:, 0:512])
                nc.gpsimd.dma_start(
                    out=xt_sb[:, c + 1, 0:512], in_=xt_r[c + 1][:, 0:512]
                )
            nc.scalar.dma_start(
                out=wq_sb[0], in_=WQ[0].rearrange("(c p) m -> p c m", p=P)
            )
            nc.scalar.dma_start(
                out=wk_sb[0], in_=WK[0].rearrange("(c p) m -> p c m", p=P)
            )
            gate = pp.tile([1, 8], F16, name="gate")
            junk = pp.tile([1, 8], F16, name="junk")

            def mk_gate(regions):
                nc.vector.tensor_add(gate, regions[0], regions[1])
                for r in regions[2:]:
                    nc.vector.tensor_add(gate, gate, r)

            def war_gate(region):
                nc.vector.tensor_add(junk, gate, region)

            def gated(eng, out, in_, region):
                war_gate(region)
                eng.dma_start(out=out, in_=in_)

            mk_gate([
                xt_sb[0:1, 6, 504:512],
                xt_sb[0:1, 7, 504:512],
                wq_sb[0][0:1, 7, 120:128],
                wk_sb[0][0:1, 7, 120:128],
            ])
            # tier 2: Wq1/Wk1 + Wv
            gated(nc.sync, wq_sb[1],
                  WQ[1].rearrange("(c p) m -> p c m", p=P),
                  wq_sb[1][0:1, 0, 0:8])
            gated(nc.gpsimd, wk_sb[1],
                  WK[1].rearrange("(c p) m -> p c m", p=P),
                  wk_sb[1][0:1, 0, 0:8])
            for c in range(0, DC, 2):
                gated(nc.sync, wv_sb[:, c, :], wv_p[:, c, :],
                      wv_sb[0:1, c, 0:8])
                gated(nc.gpsimd, wv_sb[:, c + 1, :], wv_p[:, c + 1, :],
                      wv_sb[0:1, c + 1, 0:8])
            mk_gate([
                wv_sb[0:1, 6, 8:16],
                wv_sb[0:1, 7, 8:16],
                wq_sb[1][0:1, 7, 120:128],
                wk_sb[1][0:1, 7, 120:128],
            ])
            # tier 3: remaining Wq/Wk pairs
            for pr in (2, 3):
                gated(nc.sync, wq_sb[pr],
                      WQ[pr].rearrange("(c p) m -> p c m", p=P),
                      wq_sb[pr][0:1, 0, 0:8])
                gated(nc.gpsimd, wk_sb[pr],
                      WK[pr].rearrange("(c p) m -> p c m", p=P),
                      wk_sb[pr][0:1, 0, 0:8])
            mk_gate([
                wq_sb[3][0:1, 7, 120:128],
                wk_sb[3][0:1, 7, 120:128],
            ])
            # tier 4: X block 1
            for c in range(DC):
                gated(nc.sync if c % 2 == 0 else nc.gpsimd,
                      xt_sb[:, c, 512:1024], xt_r[c][:, 512:1024],
                      xt_sb[0:1, c, 512:520])
            mk_gate([
                xt_sb[0:1, 6, 1016:1024],
                xt_sb[0:1, 7, 1016:1024],
            ])
            # tier 5: X block 2
            for c in range(DC):
                gated(nc.sync if c % 2 == 0 else nc.gpsimd,
                      xt_sb[:, c, 1024:1536], xt_r[c][:, 1024:1536],
                      xt_sb[0:1, c, 1024:1032])
            # Wo + X block 3 ride the scalar queue, but their triggers are
            # emitted later (at segment boundaries) so they never block
            # the exp stream and never compete with the critical set.
            scalar_dmas = [
                lambda: nc.scalar.dma_start(
                    out=wo_sb[:, 0:2, :], in_=wo_p[:, 0:2, :]
                ),
                lambda: nc.scalar.dma_start(
                    out=wo_sb[:, 2:4, :], in_=wo_p[:, 2:4, :]
                ),
            ] + [
                (lambda c=c: nc.scalar.dma_start(
                    out=xt_sb[:, c, 1536:2048], in_=xt_r[c][:, 1536:2048]
                ))
                for c in range(DC)
            ]

            nc.vector.memset(warm_sb, 0.0)
            nc.vector.memset(tri_sb, 1.0)
            nc.gpsimd.affine_select(
                out=tri_sb,
                in_=tri_sb,
                compare_op=mybir.AluOpType.is_ge,
                fill=0.0,
                base=0,
                channel_multiplier=-1,
                pattern=[[1, P]],
            )
            # ones column (index 64 of each head's 65-wide slot)
            v_slots = v_sb.rearrange("p s (h x) -> p s h x", x=K + 1)
            nc.vector.memset(v_slots[:, :, :, K : K + 1], 1.0)

            with (
                tc.tile_pool(name="attn", bufs=1) as ap_,
                tc.tile_pool(name="psa", bufs=1, space="PSUM") as psa,
            ):
                # ---------------- op builders -------------------------
                def v_group_ops(s):
                    """V projection for one key tile: 8 mms + 1 cast."""
                    holder = {}

                    def mm(c):
                        def f():
                            if "ps" not in holder:
                                holder["ps"] = psa.tile(
                                    [P, HG * K], F32, tag="mm", bufs=2,
                                    name=f"vps{s}",
                                )
                            nc.tensor.matmul(
                                holder["ps"],
                                xt_sb[:, c, s * P : (s + 1) * P],
                                wv_sb[:, c, :],
                                start=(c == 0),
                                stop=(c == DC - 1),
                            )
                        return f

                    def fin():
                        nc.vector.tensor_copy(
                            v_slots[:, s, :, 0:K],
                            holder["ps"].rearrange("p (h k) -> p h k", k=K),
                        )

                    return [(mm(c), MM_NS * 512) for c in range(DC)] + [(fin, 0.0)]

                def proj_unit_ops(pr, tt, which):
                    """Q or K projection for (pair, query tile): 8 mms+cast."""
                    w_sb = wq_sb[pr] if which == "q" else wk_sb[pr]
                    dst = q_sb[pr] if which == "q" else k_sb[pr]
                    holder = {}

                    def mm(c):
                        def f():
                            if "ps" not in holder:
                                holder["ps"] = psa.tile(
                                    [P, 512], F32, tag="mm", bufs=2,
                                    name=f"{which}ps{pr}_{tt}",
                                )
                            nc.tensor.matmul(
                                holder["ps"],
                                w_sb[:, c, :],
                                xt_sb[:, c, tt * 512 : (tt + 1) * 512],
                                start=(c == 0),
                                stop=(c == DC - 1),
                            )
                        return f

                    def fin():
                        nc.vector.tensor_copy(
                            dst[:, tt * 512 : (tt + 1) * 512], holder["ps"]
                        )

                    return [(mm(c), MM_NS * 512) for c in range(DC)] + [(fin, 0.0)]

                out_q = [nc.gpsimd, nc.sync]
                out_qi = [0]
                flush_mode = [False]

                def op_group_ops(t16, oc):
                    """Output-projection group for one [128 t, 512 oc] tile."""
                    holder = {}

                    def mm(s4):
                        def f():
                            if "ps" not in holder:
                                holder["ps"] = psa.tile(
                                    [P, 512], F32, tag="mm", bufs=2,
                                    name=f"ops{t16}_{oc}",
                                )
                            nc.tensor.matmul(
                                holder["ps"],
                                concat_sb[:, s4, t16 * P : (t16 + 1) * P],
                                wo_sb[:, s4, oc * 512 : (oc + 1) * 512],
                                start=(s4 == 0),
                                stop=(s4 == NPAIR - 1),
                            )
                        return f

                    def fin():
                        st_o = ap_.tile(
                            [P, 512], F16, tag="outst", bufs=6,
                            name=f"ost{t16}_{oc}",
                        )
                        if flush_mode[0] and (t16 + oc) % 2 == 0:
                            nc.scalar.copy(st_o, holder["ps"])
                        else:
                            nc.vector.tensor_copy(st_o, holder["ps"])
                        eng = out_q[out_qi[0] % len(out_q)]
                        out_qi[0] += 1
                        eng.dma_start(
                            out=OUT[
                                t16 * P : (t16 + 1) * P,
                                oc * 512 : (oc + 1) * 512,
                            ],
                            in_=st_o,
                        )

                    return [(mm(s4), MM_NS * 512) for s4 in range(NPAIR)] + [
                        (fin, 0.0)
                    ]

                # ---------------- fill queue --------------------------
                # entries [deadline_seg, cost_ns, ready_ns, kind, fn]
                # kind "pre": must run before the deadline segment's
                # scores (projections); kind "av": before its first AV
                # (V tiles); kind "op": no deadline (out-proj).
                fillq = []

                def fill_extend(deadline, ops, ready=0.0, kind="pre"):
                    for fn, cost in ops:
                        fillq.append([deadline, cost, ready, kind, fn])

                clock = {"pe": 11000.0, "sc": 11000.0}
                sc_done = {}
                exp_hist = [0.0, 0.0]  # completion of last two exps (global)

                def pop_fill():
                    """Emit the first fill op whose data has landed."""
                    for idx in range(min(len(fillq), 24)):
                        if fillq[idx][2] <= clock["pe"]:
                            _, cost, _, _, fn = fillq.pop(idx)
                            fn()
                            clock["pe"] += cost
                            return True
                    return False

                def pace_to(target):
                    while clock["pe"] < target:
                        if not pop_fill():
                            clock["pe"] = target
                            break

                def force_pop(n):
                    for _ in range(n):
                        if not fillq:
                            return
                        _, cost, _, _, fn = fillq.pop(0)
                        fn()
                        clock["pe"] += cost

                def flush_due(seg, kinds=("pre",)):
                    idx = 0
                    while idx < len(fillq):
                        dl, cost, _, kind, fn = fillq[idx]
                        if dl <= seg and kind in kinds:
                            fillq.pop(idx)
                            fn()
                            clock["pe"] += cost
                        else:
                            idx += 1

                # ---------------- attention pieces --------------------
                def score_exp(pr, tt, si):
                    m = si - 4 * tt
                    off = max(m, 0) * P
                    nv = 512 - off
                    st = psa.tile([P, 2, 512], F32, tag="stw", bufs=2)
                    ex = ap_.tile(
                        [P, 2, 512], F16, tag="exp", bufs=8,
                        name=f"exp{pr}_{tt}_{si}",
                    )
                    for h in range(2):
                        lo, hi = h * K, (h + 1) * K
                        nc.tensor.matmul(
                            st[:, h, 0:nv],
                            k_sb[pr][lo:hi, si * P : (si + 1) * P],
                            q_sb[pr][lo:hi, tt * 512 + off : (tt + 1) * 512],
                            start=True,
                            stop=True,
                            tile_position=(lo, 0),
                        )
                    clock["pe"] += MM_NS * nv + PAIR_FIX
                    nc.scalar.activation(
                        ex[:, :, 0:nv], st[:, :, 0:nv],
                        mybir.ActivationFunctionType.Exp,
                        scale=0.125,
                    )
                    start = max(clock["sc"], clock["pe"] + EXP_LAG)
                    clock["sc"] = start + 2 * nv * EXP_COL + EXP_FIX
                    sc_done[(pr, tt, si)] = clock["sc"]
                    exp_hist.append(clock["sc"])
                    if m >= 0:  # mask both heads' leading triangles
                        nc.vector.tensor_mul(
                            ex[:, :, 0:P],
                            ex[:, :, 0:P],
                            tri_sb.unsqueeze(1).broadcast_to([P, 2, P]),
                        )
                    return ex, nv, off

                def av_pair(pr, tt, si, ex, nv, off, n_s):
                    for h in range(2):
                        slot = (2 * pr + h) * (K + 1)
                        nc.tensor.matmul(
                            avs[h][:, off:512],
                            v_sb[:, si, slot : slot + K + 1],
                            ex[:, h, 0:nv],
                            start=(si == 0),
                            stop=(si == n_s - 1),
                        )
                    clock["pe"] += 2 * MM_NS * nv

                def direct_normalize(pr, tt, avs):
                    for h in range(2):
                        cols = slice(tt * 512, (tt + 1) * 512)
                        sums = ap_.tile([1, 512], F32, tag="sums", bufs=6)
                        nc.vector.tensor_copy(sums, avs[h][K : K + 1, :])
                        recip = ap_.tile([1, 512], F32, tag="recip", bufs=6)
                        nc.vector.reciprocal_approx_fast(recip, sums)
                        bc_sb = ap_.tile([K, 512], F32, tag="bc_sb", bufs=6)
                        nc.gpsimd.partition_broadcast(bc_sb, recip)
                        if h == 0:
                            dst = concat_sb[0:K, pr, cols]
                        else:
                            dst = ap_.tile([K, 512], F16, tag="tmpb", bufs=6)
                        nc.vector.tensor_mul(dst, avs[h][0:K, :], bc_sb)
                        if h == 1:
                            nc.gpsimd.dma_start(
                                out=concat_sb[K:P, pr, cols], in_=dst
                            )

                def slice_normalize(pr, tt, avs, i16):
                    """128-col slice normalize for the phase's last pair,
                    so each out-proj group unlocks as early as possible."""
                    cols_lo = i16 * P
                    for h in range(2):
                        cols = slice(tt * 512 + cols_lo, tt * 512 + cols_lo + P)
                        psl = slice(cols_lo, cols_lo + P)
                        sums = ap_.tile([1, P], F32, tag="sums", bufs=6)
                        nc.vector.tensor_copy(sums, avs[h][K : K + 1, psl])
                        recip = ap_.tile([1, P], F32, tag="recip", bufs=6)
                        nc.vector.reciprocal_approx_fast(recip, sums)
                        bc_sb = ap_.tile([K, P], F32, tag="bc_sb", bufs=6)
                        nc.gpsimd.partition_broadcast(bc_sb, recip)
                        if h == 0:
                            dst = concat_sb[0:K, pr, cols]
                        else:
                            dst = ap_.tile([K, P], F16, tag="tmpb", bufs=6)
                        nc.vector.tensor_mul(dst, avs[h][0:K, psl], bc_sb)
                        if h == 1:
                            (nc.gpsimd if i16 % 2 == 0 else nc.sync).dma_start(
                                out=concat_sb[K:P, pr, cols], in_=dst
                            )

                # ---------------- startup -----------------------------
                warm_ps = psa.tile([P, 512], F32, tag="mm", bufs=2, name="warm")

                def warm(n):
                    for _ in range(n):
                        nc.tensor.matmul(
                            warm_ps, warm_sb[:, 0:P], warm_sb,
                            start=True, stop=True,
                        )

                warm(6)
                # p0's projections emitted directly in DMA-arrival order
                # (chunks alternate between the two fast queues); scores
                # for (p0, tt0) then start while everything else streams.
                for which in ("q", "k"):
                    ops = proj_unit_ops(0, 0, which)
                    for c in (0, 4, 1, 5, 2, 6, 3, 7):
                        fn, cost = ops[c]
                        fn()
                        clock["pe"] += cost
                    ops[DC][0]()  # cast

                # fill inventory, deadline = segment index (tt*4+pr).
                # ready_ns = rough DMA landing estimate for the op's data.
                for s in range(4):
                    fill_extend(0, v_group_ops(s), ready=21500, kind="av")
                for pr in range(1, NPAIR):
                    rdy = {1: 21000, 2: 26500, 3: 27500}[pr]
                    fill_extend(pr, proj_unit_ops(pr, 0, "q"), ready=rdy)
                    fill_extend(pr, proj_unit_ops(pr, 0, "k"), ready=rdy)
                for tt in range(1, NT):
                    xt_rdy = [0, 31500, 37000, 42000][tt]
                    for s in range(4 * tt, 4 * tt + 4):
                        fill_extend(
                            4 * tt, v_group_ops(s), ready=xt_rdy, kind="av"
                        )
                    for pr in range(NPAIR):
                        fill_extend(
                            4 * tt + pr, proj_unit_ops(pr, tt, "q"), ready=xt_rdy
                        )
                        fill_extend(
                            4 * tt + pr, proj_unit_ops(pr, tt, "k"), ready=xt_rdy
                        )

                # ---------------- main tt-major loop ------------------
                for tt in range(NT):
                    n_s = 4 * tt + 4
                    for pr in range(NPAIR):
                        seg = 4 * tt + pr
                        flush_due(seg)
                        if seg >= 1:
                            for _ in range(3):
                                if scalar_dmas:
                                    scalar_dmas.pop(0)()
                        avs = [
                            psa.tile(
                                [K + 1, 512], F32, tag="av", bufs=2,
                                name=f"av{pr}_{tt}_{h2}",
                            )
                            for h2 in range(2)
                        ]
                        # AV trails the score stream by 2 steps so an AV
                        # at the PE queue head never waits on its exp.
                        pend = {}
                        for si in range(n_s):
                            # score pair needs the st slot freed by the
                            # exp two score-pairs back (global rotation)
                            pace_to(exp_hist[-2] + 100)
                            pend[si] = score_exp(pr, tt, si)
                            if si == 1:
                                # V tiles must be in the stream before
                                # the segment's first AV
                                flush_due(seg, kinds=("pre", "av"))
                            if si >= 2:
                                pace_to(sc_done[(pr, tt, si - 2)] + 80)
                                av_pair(pr, tt, si - 2, *pend.pop(si - 2), n_s)
                        for sj in (n_s - 2, n_s - 1):
                            pace_to(sc_done[(pr, tt, sj)] + 80)
                            av_pair(pr, tt, sj, *pend.pop(sj), n_s)
                        if pr == NPAIR - 1:
                            last = tt == NT - 1
                            if last:
                                flush_mode[0] = True
                            for i16 in range(4):
                                slice_normalize(pr, tt, avs, i16)
                                t16 = 4 * tt + i16
                                # reserve two late-tt2 groups as flush
                                # filler so the HAM clock stays up while
                                # the final normalizes drain
                                rsv = float("inf") if tt == 2 and i16 >= 2 else 0.0
                                fill_extend(99, op_group_ops(t16, 0))
                                fill_extend(99, op_group_ops(t16, 1), ready=rsv)
                                if last:
                                    # keep the PE activity window dense
                                    # while the concat bounce flies so
                                    # the HAM clock stays at 8/8 through
                                    # the flush
                                    warm(2)
                                    force_pop(5)
                                    warm(1)
                                    force_pop(5)
                        else:
                            direct_normalize(pr, tt, avs)

                # ---------------- flush (force-pop, ignore ready) -----
                flush_mode[0] = True
                while fillq:
                    _, cost, _, _, fn = fillq.pop(0)
                    fn()
                    clock["pe"] += cost
    _fuse_score_ldweights(nc)
    nc.compile()
    return nc


def _fuse_score_ldweights(nc):
    """Merge each score pair's two 64-row LDWEIGHTS into one 128-row load.

    The post-Tile IR carries [Ldw(h0 64p), MM(0,0), Ldw(h1 64p), MM(64,0)]
    per key tile. With two LDWs the PE stalls ~100ns on each side of the
    pair (single background weight buffer). One 128-row LDW loads both
    heads' K slice at once; the row-tiled matmuls then address their own
    row groups of the already-loaded array.
    """
    fn = list(nc.m.functions)[0]
    fused = 0
    for blk in fn.blocks:
        insts = blk.instructions
        # pattern-match on the PE-engine subsequence: other engines'
        # instructions interleave freely in the block list
        pe = [
            (i, x)
            for i, x in enumerate(insts)
            if type(x).__name__ in ("InstLdweights", "InstMatmult")
        ]
        drop = []
        for k in range(len(pe) - 3):
            (_, a), (_, b), (ic, c), (_, d) = pe[k], pe[k + 1], pe[k + 2], pe[k + 3]
            if not (
                type(a).__name__ == "InstLdweights"
                and type(b).__name__ == "InstMatmult"
                and type(c).__name__ == "InstLdweights"
                and type(d).__name__ == "InstMatmult"
            ):
                continue
            if not (
                tuple(b.tile_size or ()) == (64, 128)
                and tuple(b.tile_position or ()) == (0, 0)
                and tuple(d.tile_size or ()) == (64, 128)
                and tuple(d.tile_position or ()) == (64, 0)
            ):
                continue
            apA, apC = a.ins[0], c.ins[0]
            pa, pc = list(apA.ap), list(apC.ap)
            if not (
                len(pa) == 2
                and pa[0][1] == 64
                and pc[0][1] == 64
                and pa[0][0] == pc[0][0]
                and pa[1] == pc[1]
                and apC.offset == apA.offset + 64 * pa[0][0]
                and c.sync_info is None
            ):
                continue
            apA.ap = [[pa[0][0], 128], pa[1]]
            if tuple(a.tile_size or ()) == (64, 128):
                a.tile_size = (128, 128)
            a.merge_dependencies_from(c)
            drop.append(ic)
            fused += 1
        for j in sorted(drop, reverse=True):
            del insts[j]
    assert fused > 0, "score LDW fusion matched nothing"


def shard_inputs(X, Wq, Wk, Wv, Wo):
    """Host-side shard prep: core c handles batch c//2, head group c%2."""
    in_maps = []
    for c in range(8):
        b, g = c // 2, c % 2
        heads = range(g * HG, (g + 1) * HG)
        wq = np.stack(
            [
                np.concatenate([Wq[g * HG + 2 * p], Wq[g * HG + 2 * p + 1]], axis=1)
                for p in range(NPAIR)
            ]
        )
        wk = np.stack(
            [
                np.concatenate([Wk[g * HG + 2 * p], Wk[g * HG + 2 * p + 1]], axis=1)
                for p in range(NPAIR)
            ]
        )
        wv = np.concatenate([Wv[h] for h in heads], axis=1)
        wo = Wo[:, g * 512 : (g + 1) * 512].T
        in_maps.append(
            {
                "xt": np.ascontiguousarray(X[b].T).astype(np.float16),
                "wq": np.ascontiguousarray(wq).astype(np.float16),
                "wk": np.ascontiguousarray(wk).astype(np.float16),
                "wv": np.ascontiguousarray(wv).astype(np.float16),
                "wo": np.ascontiguousarray(wo).astype(np.float16),
            }
        )
    return in_maps


_MODULE = None


def _get_module():
    global _MODULE
    if _MODULE is None:
        _MODULE = build_module()
    return _MODULE


def kernel(X, Wq, Wk, Wv, Wo, bo, _want_results=None):
    from concourse.bass_utils import run_bass_kernel_spmd

    nc = _get_module()
    in_maps = shard_inputs(
        np.asarray(X), np.asarray(Wq), np.asarray(Wk), np.asarray(Wv), np.asarray(Wo)
    )
    res = run_bass_kernel_spmd(nc, in_maps, core_ids=list(range(8)))
    if _want_results is not None:
        _want_results.append(res)
    out = np.empty((B, T, H * K), dtype=np.float32)
    bo = np.asarray(bo, dtype=np.float32)
    for b in range(B):
        out[b] = (
            res.results[2 * b]["out"].astype(np.float32)
            + res.results[2 * b + 1]["out"].astype(np.float32)
            + bo
        )
    return out
